# revision 11
# baseline (speedup 1.0000x reference)
"""Multi-head attention (dense_transformer) on 8 TRN2 NeuronCores.

Decomposition (zero collectives): core c handles batch b = c//2 and query
half qh = c%2.  Each core computes K/V for its batch's full 2048 tokens
(replicated across the 2 cores sharing a batch -- cheaper than any on-chip
collective here), Q for its own 1024 query tokens, all 16 attention heads,
and the output projection for its tokens.  Host does the sharding, the
layout transposes, and the bf16 casts; the NEFF sees only matmul-friendly
layouts.

Layouts on chip (transposed-activation style):
  qT/kT:  [odim (partitions), token (free)]   <- lhsT = w_qkv[c, odim]
  v_aug:  [token%128, token//128, head, 65]   (natural v + ones column;
          the ones column makes the softmax denominator fall out of the
          attn@v matmul as psum row 64)
  scoresT[kt, qt] -> exp on ScalarE (SCALE folded into the activation)
  attn@v: out[d(+den), qt] accumulated over kt tiles
  normalize: reciprocal(den) -> K=1 ones-matmul broadcast -> VectorE
  proj:   yT[odim, qt] = w_proj[c, odim].T @ attnoutT[c, qt]
"""

import sys

if "/opt/trn_rl_repo" not in sys.path:
    sys.path.insert(0, "/opt/trn_rl_repo")

import numpy as np
import ml_dtypes

import concourse.bass as bass
import concourse.mybir as mybir
from concourse.tile import TileContext
from concourse.bass_utils import run_bass_kernel_spmd

F32 = mybir.dt.float32
BF16 = mybir.dt.bfloat16

B = 4
N = 2048
C = 1024
H = 16
D = 64
SCALE = D**-0.5
NQ = N // 2  # query tokens per core
NCORES = 8

CT = C // 128  # 8 c-tiles
TOK_CHUNK = 512
N_CHUNKS = N // TOK_CHUNK  # 4
Q_CHUNKS = NQ // TOK_CHUNK  # 2
KT_TILES = N // 128  # 16

_DMA_TYPES = ("DMA", "Collective", "TriggeredCopy")


def _split_sync_waits(nc, max_waits: int = 1) -> int:
    """This container's walrus rejects TPB instructions with >1 sync-wait;
    hoist extras onto InstNoOps inserted just before, on the same engine."""
    n_split = 0
    for fn in nc.m.functions:
        for block in fn.blocks:
            out = []
            changed = False
            for inst in block.instructions:
                tname = type(inst).__name__
                si = getattr(inst, "sync_info", None)
                if si is not None and len(si.on_wait) > max_waits:
                    waits = list(si.on_wait)
                    n_extra = len(waits) - max_waits
                    for i in range(0, n_extra, max_waits):
                        out.append(
                            mybir.InstNoOp(
                                name=f"{inst.name}-sw{i}",
                                sync_info=mybir.SyncInfo(
                                    on_wait=waits[i : i + max_waits], on_update=[]
                                ),
                                bass_nofuse=True,
                                engine=inst.engine,
                            )
                        )
                    inst.sync_info = mybir.SyncInfo(
                        on_wait=waits[n_extra:], on_update=list(si.on_update)
                    )
                    changed = True
                    n_split += 1
                out.append(inst)
            if changed:
                block.instructions = out
    return n_split


def build(split=True, n_chunks=N_CHUNKS, q_chunks=1, n_heads=H, kt_tiles=KT_TILES):
    nc = bass.Bass(target_bir_lowering=False)

    xT_ext = nc.declare_dram_parameter("xT", [C, N], BF16, isOutput=False)
    xqT_ext = nc.declare_dram_parameter("xqT", [C, NQ], BF16, isOutput=False)
    wqkv_ext = nc.declare_dram_parameter("w_qkv", [C, 3 * C], BF16, isOutput=False)
    wproj_ext = nc.declare_dram_parameter("w_proj", [C, C], BF16, isOutput=False)
    bq_ext = nc.declare_dram_parameter("b_q", [C, 1], F32, isOutput=False)
    bk_ext = nc.declare_dram_parameter("b_k", [C, 1], F32, isOutput=False)
    bv0_ext = nc.declare_dram_parameter("b_v0", [D, H], F32, isOutput=False)
    bp_ext = nc.declare_dram_parameter("b_p", [C, 1], F32, isOutput=False)
    out_ext = nc.declare_dram_parameter("out", [C, NQ], F32, isOutput=True)

    xT_r = xT_ext[:].rearrange("(o p) n -> p o n", p=128)
    xqT_r = xqT_ext[:].rearrange("(o p) n -> p o n", p=128)
    out_r = out_ext[:].rearrange("(o p) n -> p o n", p=128)

    with TileContext(nc) as tc:
        with (
            tc.tile_pool(name="const", bufs=1) as const,
            tc.tile_pool(name="xin", bufs=2) as xin,
            tc.tile_pool(name="big", bufs=1) as big,
            tc.tile_pool(name="attn", bufs=3) as attnp,
            tc.tile_pool(name="ao", bufs=1) as aop,
            tc.tile_pool(name="small", bufs=2) as small,
            tc.tile_pool(name="ost", bufs=2) as ostp,
            tc.tile_pool(name="ps_mm", bufs=2, space="PSUM") as ps_mm,
            tc.tile_pool(name="ps_s", bufs=2, space="PSUM") as ps_s,
            tc.tile_pool(name="ps_av", bufs=1, space="PSUM") as ps_av,
        ):
            wqkv = const.tile([128, CT, 3 * C], BF16)
            wproj = const.tile([128, CT, C], BF16)
            bq = const.tile([128, CT], F32)
            bk = const.tile([128, CT], F32)
            bv0 = const.tile([D, H], F32)
            bp = const.tile([128, CT], F32)
            ones_row = const.tile([1, D], F32)

            kT = big.tile([128, CT, N], BF16)
            qT = big.tile([128, CT, NQ], BF16)
            v_aug = big.tile([128, KT_TILES, H, D + 1], BF16)

            nc.sync.dma_start(out=wqkv[:], in_=wqkv_ext[:].rearrange("(o p) n -> p o n", p=128))
            nc.sync.dma_start(out=wproj[:], in_=wproj_ext[:].rearrange("(o p) n -> p o n", p=128))
            nc.sync.dma_start(out=bq[:], in_=bq_ext[:].rearrange("(o p) 1 -> p o", p=128))
            nc.sync.dma_start(out=bk[:], in_=bk_ext[:].rearrange("(o p) 1 -> p o", p=128))
            nc.sync.dma_start(out=bv0[:], in_=bv0_ext[:])
            nc.sync.dma_start(out=bp[:], in_=bp_ext[:].rearrange("(o p) 1 -> p o", p=128))
            nc.vector.memset(ones_row[:], 1.0)
            for h in range(H):
                nc.vector.memset(v_aug[:, :, h, D : D + 1], 1.0)

            # ---- Phase B: qkv projections -------------------------------
            for t in range(n_chunks):
                x_c = xin.tile([128, CT, TOK_CHUNK], BF16, tag="xc")
                nc.sync.dma_start(
                    out=x_c[:], in_=xT_r[:, :, t * TOK_CHUNK : (t + 1) * TOK_CHUNK]
                )
                # kT for this token chunk
                for m in range(CT):
                    ps = ps_mm.tile([128, TOK_CHUNK], F32)
                    for kc in range(CT):
                        nc.tensor.matmul(
                            ps[:],
                            lhsT=wqkv[:, kc, C + m * 128 : C + (m + 1) * 128],
                            rhs=x_c[:, kc, :],
                            start=(kc == 0),
                            stop=(kc == CT - 1),
                        )
                    nc.vector.tensor_tensor(
                        kT[:, m, t * TOK_CHUNK : (t + 1) * TOK_CHUNK],
                        ps[:],
                        bk[:, m : m + 1].to_broadcast([128, TOK_CHUNK]),
                        mybir.AluOpType.add,
                    )
                # v (natural layout) for this token chunk
                for tt in range(TOK_CHUNK // 128):
                    kt_idx = t * (TOK_CHUNK // 128) + tt
                    for vc in range(2):
                        ps = ps_mm.tile([128, TOK_CHUNK], F32)
                        for kc in range(CT):
                            nc.tensor.matmul(
                                ps[:],
                                lhsT=x_c[:, kc, tt * 128 : (tt + 1) * 128],
                                rhs=wqkv[:, kc, 2 * C + vc * 512 : 2 * C + (vc + 1) * 512],
                                start=(kc == 0),
                                stop=(kc == CT - 1),
                            )
                        nc.vector.tensor_copy(
                            v_aug[:, kt_idx, vc * 8 : (vc + 1) * 8, 0:D],
                            ps[:].rearrange("p (h d) -> p h d", d=D),
                        )
            # qT for this core's query tokens
            for tq in range(NQ // TOK_CHUNK):
                xq_c = xin.tile([128, CT, TOK_CHUNK], BF16, tag="xc")
                nc.sync.dma_start(
                    out=xq_c[:], in_=xqT_r[:, :, tq * TOK_CHUNK : (tq + 1) * TOK_CHUNK]
                )
                for m in range(CT):
                    ps = ps_mm.tile([128, TOK_CHUNK], F32)
                    for kc in range(CT):
                        nc.tensor.matmul(
                            ps[:],
                            lhsT=wqkv[:, kc, m * 128 : (m + 1) * 128],
                            rhs=xq_c[:, kc, :],
                            start=(kc == 0),
                            stop=(kc == CT - 1),
                        )
                    nc.vector.tensor_tensor(
                        qT[:, m, tq * TOK_CHUNK : (tq + 1) * TOK_CHUNK],
                        ps[:],
                        bq[:, m : m + 1].to_broadcast([128, TOK_CHUNK]),
                        mybir.AluOpType.add,
                    )

            # ---- Phase C: attention + projection (single 1024-wide chunk) ----
            # PE emission is software-pipelined one kt-stage ahead of ACT so
            # the attn@v matmul never stalls the PE waiting on the exp.
            for Q in range(q_chunks):
                ao = aop.tile([128, CT, NQ], BF16)
                at_tiles = {}

                def scores_stage(h, kt):
                    pb = (h % 2) * D
                    mt = h // 2
                    pss = ps_s.tile([128, NQ], F32)
                    for half in range(2):
                        nc.tensor.matmul(
                            pss[:, half * 512 : (half + 1) * 512],
                            lhsT=kT[pb : pb + D, mt, kt * 128 : (kt + 1) * 128],
                            rhs=qT[pb : pb + D, mt, half * 512 : (half + 1) * 512],
                            start=True,
                            stop=True,
                        )
                    at = attnp.tile([128, NQ], BF16)
                    nc.scalar.activation(
                        at[:],
                        pss[:],
                        mybir.ActivationFunctionType.Exp,
                        scale=float(SCALE),
                    )
                    at_tiles[(h, kt)] = at

                def av_stage(h, kt, pav):
                    at = at_tiles.pop((h, kt))
                    for half in range(2):
                        nc.tensor.matmul(
                            pav[0 : D + 1, half * 512 : (half + 1) * 512],
                            lhsT=v_aug[:, kt, h, :],
                            rhs=at[:, half * 512 : (half + 1) * 512],
                            start=(kt == 0),
                            stop=(kt == kt_tiles - 1),
                        )

                pavs = {}
                for h in range(n_heads):
                    pavs[h] = ps_av.tile([128, NQ], F32, name="pav", tag="pav")
                    scores_stage(h, 0)
                    for kt in range(1, kt_tiles):
                        scores_stage(h, kt)
                        av_stage(h, kt - 1, pavs[h])
                    av_stage(h, kt_tiles - 1, pavs[h])

                    pb = (h % 2) * D
                    mt = h // 2
                    pav = pavs[h]
                    av_sb = small.tile([128, NQ], F32, tag="av")
                    nc.vector.tensor_copy(av_sb[0 : D + 1, :], pav[0 : D + 1, :])
                    rrow = small.tile([1, NQ], F32, tag="rrow")
                    nc.vector.reciprocal(rrow[:], av_sb[D : D + 1, :])
                    for half in range(2):
                        hsl = slice(half * 512, (half + 1) * 512)
                        pbc = ps_mm.tile([128, 512], F32, tag="ps", name="pbc")
                        nc.tensor.matmul(
                            pbc[0:D, :],
                            lhsT=ones_row[:],
                            rhs=rrow[:, hsl],
                            start=True,
                            stop=True,
                        )
                        t1 = small.tile([D, 512], F32, tag="t1")
                        nc.vector.tensor_tensor(
                            t1[:], av_sb[0:D, hsl], pbc[0:D, :], mybir.AluOpType.mult
                        )
                        nc.vector.tensor_tensor(
                            ao[pb : pb + D, mt, hsl],
                            t1[:],
                            bv0[:, h : h + 1].to_broadcast([D, 512]),
                            mybir.AluOpType.add,
                        )
                # projection
                for od in range(CT):
                    for half in range(2):
                        hsl = slice(half * 512, (half + 1) * 512)
                        ps = ps_mm.tile([128, 512], F32)
                        for kc in range(CT):
                            nc.tensor.matmul(
                                ps[:],
                                lhsT=wproj[:, kc, od * 128 : (od + 1) * 128],
                                rhs=ao[:, kc, hsl],
                                start=(kc == 0),
                                stop=(kc == CT - 1),
                            )
                        o_st = ostp.tile([128, 512], F32)
                        nc.vector.tensor_tensor(
                            o_st[:],
                            ps[:],
                            bp[:, od : od + 1].to_broadcast([128, 512]),
                            mybir.AluOpType.add,
                        )
                        nc.sync.dma_start(out=out_r[:, od, hsl], in_=o_st[:])

    if split:
        _split_sync_waits(nc)
    return nc


_CACHED_NC = None


def _get_nc():
    global _CACHED_NC
    if _CACHED_NC is None:
        _CACHED_NC = build()
    return _CACHED_NC


def make_in_maps(x, w_qkv, b_qkv, w_proj, b_proj):
    bf = ml_dtypes.bfloat16
    wq = np.ascontiguousarray(w_qkv.astype(bf))
    wp = np.ascontiguousarray(w_proj.astype(bf))
    b_q = np.ascontiguousarray(b_qkv[0:C].reshape(C, 1).astype(np.float32))
    b_k = np.ascontiguousarray(b_qkv[C : 2 * C].reshape(C, 1).astype(np.float32))
    b_v0 = np.ascontiguousarray(
        b_qkv[2 * C : 3 * C].reshape(H, D).T.astype(np.float32)
    )
    b_p = np.ascontiguousarray(b_proj.reshape(C, 1).astype(np.float32))

    in_maps = []
    for core in range(NCORES):
        b = core // 2
        qh = core % 2
        xb = x[b]  # [N, C] f32
        xT = np.ascontiguousarray(xb.T.astype(bf))  # [C, N]
        xqT = np.ascontiguousarray(
            xb[qh * NQ : (qh + 1) * NQ].T.astype(bf)
        )  # [C, NQ]
        in_maps.append(
            {
                "xT": xT,
                "xqT": xqT,
                "w_qkv": wq,
                "w_proj": wp,
                "b_q": b_q,
                "b_k": b_k,
                "b_v0": b_v0,
                "b_p": b_p,
            }
        )
    return in_maps


def run(x, w_qkv, b_qkv, w_proj, b_proj, trace=False, **spmd_kwargs):
    nc = _get_nc()
    in_maps = make_in_maps(x, w_qkv, b_qkv, w_proj, b_proj)
    res = run_bass_kernel_spmd(
        nc, in_maps, core_ids=list(range(NCORES)), trace=trace, **spmd_kwargs
    )
    out = np.empty((B, N, C), dtype=np.float32)
    for core in range(NCORES):
        b = core // 2
        qh = core % 2
        yT = res.results[core]["out"]  # [C, NQ] f32
        out[b, qh * NQ : (qh + 1) * NQ, :] = yT.T
    return out, res


def kernel(x, w_qkv, b_qkv, w_proj, b_proj):
    x = np.asarray(x, dtype=np.float32)
    w_qkv = np.asarray(w_qkv, dtype=np.float32)
    b_qkv = np.asarray(b_qkv, dtype=np.float32)
    w_proj = np.asarray(w_proj, dtype=np.float32)
    b_proj = np.asarray(b_proj, dtype=np.float32)
    out, _ = run(x, w_qkv, b_qkv, w_proj, b_proj, trace=False)
    return out


# revision 20
# speedup vs baseline: 1.1483x; 1.1483x over previous
"""Multi-head attention (dense_transformer) on 8 TRN2 NeuronCores.

Decomposition (zero collectives): core c handles batch b = c//2 and query
half qh = c%2.  Each core computes K/V for its batch's full 2048 tokens
(replicated across the 2 cores sharing a batch -- cheaper than any on-chip
collective here), Q for its own 1024 query tokens, all 16 attention heads,
and the output projection for its tokens.  Host does the sharding, the
layout transposes, and the bf16 casts; the NEFF sees only matmul-friendly
layouts.

Layouts on chip (transposed-activation style):
  qT/kT:  [odim (partitions), token (free)]   <- lhsT = w_qkv[c, odim]
  v_aug:  [token%128, token//128, head, 65]   (natural v + ones column;
          the ones column makes the softmax denominator fall out of the
          attn@v matmul as psum row 64)
  scoresT[kt, qt] -> exp on ScalarE (SCALE folded into the activation)
  attn@v: out[d(+den), qt] accumulated over kt tiles
  normalize: reciprocal(den) -> K=1 ones-matmul broadcast -> VectorE
  proj:   yT[odim, qt] = w_proj[c, odim].T @ attnoutT[c, qt]
"""

import sys

if "/opt/trn_rl_repo" not in sys.path:
    sys.path.insert(0, "/opt/trn_rl_repo")

import numpy as np
import ml_dtypes

import concourse.bass as bass
import concourse.mybir as mybir
from concourse.tile import TileContext
from concourse.bass_utils import run_bass_kernel_spmd

F32 = mybir.dt.float32
BF16 = mybir.dt.bfloat16

B = 4
N = 2048
C = 1024
H = 16
D = 64
SCALE = D**-0.5
NQ = N // 2  # query tokens per core
NCORES = 8

CT = C // 128  # 8 c-tiles
TOK_CHUNK = 512
N_CHUNKS = N // TOK_CHUNK  # 4
Q_CHUNKS = NQ // TOK_CHUNK  # 2
KT_TILES = N // 128  # 16

_DMA_TYPES = ("DMA", "Collective", "TriggeredCopy")


def _split_sync_waits(nc, max_waits: int = 1) -> int:
    """This container's walrus rejects TPB instructions with >1 sync-wait;
    hoist extras onto InstNoOps inserted just before, on the same engine."""
    n_split = 0
    for fn in nc.m.functions:
        for block in fn.blocks:
            out = []
            changed = False
            for inst in block.instructions:
                tname = type(inst).__name__
                si = getattr(inst, "sync_info", None)
                if si is not None and len(si.on_wait) > max_waits:
                    waits = list(si.on_wait)
                    n_extra = len(waits) - max_waits
                    for i in range(0, n_extra, max_waits):
                        out.append(
                            mybir.InstNoOp(
                                name=f"{inst.name}-sw{i}",
                                sync_info=mybir.SyncInfo(
                                    on_wait=waits[i : i + max_waits], on_update=[]
                                ),
                                bass_nofuse=True,
                                engine=inst.engine,
                            )
                        )
                    inst.sync_info = mybir.SyncInfo(
                        on_wait=waits[n_extra:], on_update=list(si.on_update)
                    )
                    changed = True
                    n_split += 1
                out.append(inst)
            if changed:
                block.instructions = out
    return n_split


def build(split=True, n_chunks=N_CHUNKS, q_chunks=1, n_heads=H, kt_tiles=KT_TILES):
    nc = bass.Bass(target_bir_lowering=False)

    xT_ext = nc.declare_dram_parameter("xT", [C, N], BF16, isOutput=False)
    xqT_ext = nc.declare_dram_parameter("xqT", [C, NQ], BF16, isOutput=False)
    wqkv_ext = nc.declare_dram_parameter("w_qkv", [C, 3 * C], BF16, isOutput=False)
    wproj_ext = nc.declare_dram_parameter("w_proj", [C, C], BF16, isOutput=False)
    bq_ext = nc.declare_dram_parameter("b_q", [C, 1], F32, isOutput=False)
    bk_ext = nc.declare_dram_parameter("b_k", [C, 1], F32, isOutput=False)
    bv0_ext = nc.declare_dram_parameter("b_v0", [D, H], F32, isOutput=False)
    bp_ext = nc.declare_dram_parameter("b_p", [C, 1], F32, isOutput=False)
    out_ext = nc.declare_dram_parameter("out", [C, NQ], F32, isOutput=True)

    xT_r = xT_ext[:].rearrange("(o p) n -> p o n", p=128)
    xqT_r = xqT_ext[:].rearrange("(o p) n -> p o n", p=128)
    out_r = out_ext[:].rearrange("(o p) n -> p o n", p=128)

    with TileContext(nc) as tc:
        with (
            tc.tile_pool(name="const", bufs=1) as const,
            tc.tile_pool(name="xin", bufs=2) as xin,
            tc.tile_pool(name="big", bufs=1) as big,
            tc.tile_pool(name="attn", bufs=3) as attnp,
            tc.tile_pool(name="ao", bufs=1) as aop,
            tc.tile_pool(name="small", bufs=2) as small,
            tc.tile_pool(name="ost", bufs=1) as ostp,
            tc.tile_pool(name="ps_s", bufs=2, space="PSUM") as ps_s,
            tc.tile_pool(name="ps_av", bufs=1, space="PSUM") as ps_av,
            tc.tile_pool(name="ps_den", bufs=1, space="PSUM") as ps_den,
        ):
            wqkv = const.tile([128, CT, 3 * C], BF16)
            wproj = const.tile([128, CT, C], BF16)
            bq = const.tile([128, CT], F32)
            bk = const.tile([128, CT], F32)
            bv0 = const.tile([D, H], F32)
            bp = const.tile([128, CT], F32)
            ones_col = const.tile([128, 1], BF16)
            e0_block = const.tile([128, D], F32)
            d0 = const.tile([128, NQ], F32)

            kT = big.tile([128, CT, N], BF16)
            qT = big.tile([128, CT, NQ], BF16)
            v64 = big.tile([128, KT_TILES, H, D], BF16)

            nc.sync.dma_start(out=wqkv[:], in_=wqkv_ext[:].rearrange("(o p) n -> p o n", p=128))
            nc.sync.dma_start(out=wproj[:], in_=wproj_ext[:].rearrange("(o p) n -> p o n", p=128))
            nc.sync.dma_start(out=bq[:], in_=bq_ext[:].rearrange("(o p) 1 -> p o", p=128))
            nc.sync.dma_start(out=bk[:], in_=bk_ext[:].rearrange("(o p) 1 -> p o", p=128))
            nc.sync.dma_start(out=bv0[:], in_=bv0_ext[:])
            nc.sync.dma_start(out=bp[:], in_=bp_ext[:].rearrange("(o p) 1 -> p o", p=128))
            nc.vector.memset(ones_col[:], 1.0)
            nc.vector.memset(e0_block[:], 0.0)
            nc.vector.memset(e0_block[0:1, :], 1.0)
            nc.vector.memset(d0[:], 0.0)

            # ---- Phase B: qkv projections -------------------------------
            for t in range(n_chunks):
                x_c = xin.tile([128, CT, TOK_CHUNK], BF16, tag="xc")
                nc.sync.dma_start(
                    out=x_c[:], in_=xT_r[:, :, t * TOK_CHUNK : (t + 1) * TOK_CHUNK]
                )
                # kT for this token chunk (two odim tiles per 2-bank psum tile)
                for m2 in range(CT // 2):
                    ps = ps_s.tile([128, NQ], F32, name="ps", tag="ps")
                    for sub in range(2):
                        m = m2 * 2 + sub
                        for kc in range(CT):
                            nc.tensor.matmul(
                                ps[:, sub * 512 : (sub + 1) * 512],
                                lhsT=wqkv[:, kc, C + m * 128 : C + (m + 1) * 128],
                                rhs=x_c[:, kc, :],
                                start=(kc == 0),
                                stop=(kc == CT - 1),
                            )
                    nc.vector.tensor_tensor(
                        kT[:, m2 * 2 : m2 * 2 + 2, t * TOK_CHUNK : (t + 1) * TOK_CHUNK],
                        ps[:].rearrange("p (s n) -> p s n", s=2),
                        bk[:, m2 * 2 : m2 * 2 + 2, None].to_broadcast([128, 2, TOK_CHUNK]),
                        mybir.AluOpType.add,
                    )
                # v (natural layout) for this token chunk
                for tt in range(TOK_CHUNK // 128):
                    kt_idx = t * (TOK_CHUNK // 128) + tt
                    ps = ps_s.tile([128, NQ], F32, name="ps", tag="ps")
                    for vc in range(2):
                        for kc in range(CT):
                            nc.tensor.matmul(
                                ps[:, vc * 512 : (vc + 1) * 512],
                                lhsT=x_c[:, kc, tt * 128 : (tt + 1) * 128],
                                rhs=wqkv[:, kc, 2 * C + vc * 512 : 2 * C + (vc + 1) * 512],
                                start=(kc == 0),
                                stop=(kc == CT - 1),
                            )
                    nc.vector.tensor_copy(
                        v64[:, kt_idx, :, :],
                        ps[:].rearrange("p (h d) -> p h d", d=D),
                    )
            # qT for this core's query tokens
            for tq in range(NQ // TOK_CHUNK):
                xq_c = xin.tile([128, CT, TOK_CHUNK], BF16, tag="xc")
                nc.sync.dma_start(
                    out=xq_c[:], in_=xqT_r[:, :, tq * TOK_CHUNK : (tq + 1) * TOK_CHUNK]
                )
                for m2 in range(CT // 2):
                    ps = ps_s.tile([128, NQ], F32, name="ps", tag="ps")
                    for sub in range(2):
                        m = m2 * 2 + sub
                        for kc in range(CT):
                            nc.tensor.matmul(
                                ps[:, sub * 512 : (sub + 1) * 512],
                                lhsT=wqkv[:, kc, m * 128 : (m + 1) * 128],
                                rhs=xq_c[:, kc, :],
                                start=(kc == 0),
                                stop=(kc == CT - 1),
                            )
                    nc.vector.tensor_tensor(
                        qT[:, m2 * 2 : m2 * 2 + 2, tq * TOK_CHUNK : (tq + 1) * TOK_CHUNK],
                        ps[:].rearrange("p (s n) -> p s n", s=2),
                        bq[:, m2 * 2 : m2 * 2 + 2, None].to_broadcast([128, 2, TOK_CHUNK]),
                        mybir.AluOpType.add,
                    )

            # ---- Phase C: attention + projection (head pairs, full-array MMs) ----
            # K=64 matmuls run at half clock AND poison neighbors, so the two
            # heads sharing an odim tile (partitions 0:64 / 64:128) are issued
            # as concurrent row-group pairs; attn@V pairs are col-packed
            # (M=64 each) into one psum tile; softmax denominators come from
            # K=128 ones-column matmuls; the reciprocal-broadcast is a K=128
            # matmul against a zero-padded row (avoids cold K=1 matmuls).
            for Q in range(q_chunks):
                ao = aop.tile([128, CT, NQ], BF16)
                for pair in range(n_heads // 2):
                    mt = pair
                    h_e, h_o = 2 * pair, 2 * pair + 1
                    pav = ps_av.tile([128, NQ], F32, name="pav", tag="pav")
                    dens = ps_den.tile([128, NQ], F32, name="dens", tag="dens")
                    for kt in range(kt_tiles):
                        pss_e = ps_s.tile([128, NQ], F32, name="pss_e", tag="ps")
                        pss_o = ps_s.tile([128, NQ], F32, name="pss_o", tag="ps")
                        for half in range(2):
                            hsl = slice(half * 512, (half + 1) * 512)
                            nc.tensor.matmul(
                                pss_e[:, hsl],
                                lhsT=kT[0:D, mt, kt * 128 : (kt + 1) * 128],
                                rhs=qT[0:D, mt, hsl],
                                start=True,
                                stop=True,
                            )
                            nc.tensor.matmul(
                                pss_o[:, hsl],
                                lhsT=kT[D:128, mt, kt * 128 : (kt + 1) * 128],
                                rhs=qT[D:128, mt, hsl],
                                start=True,
                                stop=True,
                            )
                        at_e = attnp.tile([128, NQ], BF16, name="at_e", tag="at")
                        at_o = attnp.tile([128, NQ], BF16, name="at_o", tag="at")
                        nc.scalar.activation(
                            at_e[:], pss_e[:],
                            mybir.ActivationFunctionType.Exp, scale=float(SCALE),
                        )
                        nc.scalar.activation(
                            at_o[:], pss_o[:],
                            mybir.ActivationFunctionType.Exp, scale=float(SCALE),
                        )
                        first, last = kt == 0, kt == kt_tiles - 1
                        for half in range(2):
                            hsl = slice(half * 512, (half + 1) * 512)
                            nc.tensor.matmul(
                                pav[0:D, hsl],
                                lhsT=v64[:, kt, h_e, :],
                                rhs=at_e[:, hsl],
                                start=first, stop=last,
                                skip_group_check=True,
                            )
                            nc.tensor.matmul(
                                pav[D:128, hsl],
                                lhsT=v64[:, kt, h_o, :],
                                rhs=at_o[:, hsl],
                                start=first, stop=last,
                                tile_position=(0, D),
                                skip_group_check=True,
                            )
                            nc.tensor.matmul(
                                dens[0:1, hsl],
                                lhsT=ones_col[:],
                                rhs=at_e[:, hsl],
                                start=first, stop=last,
                                skip_group_check=True,
                            )
                            nc.tensor.matmul(
                                dens[32:33, hsl],
                                lhsT=ones_col[:],
                                rhs=at_o[:, hsl],
                                start=first, stop=last,
                                tile_position=(0, 32),
                                skip_group_check=True,
                            )
                    # normalize both heads of the pair
                    av_sb = small.tile([128, NQ], F32, tag="av")
                    nc.vector.tensor_copy(av_sb[:], pav[:])
                    for par, h_cur in ((0, h_e), (1, h_o)):
                        nc.vector.tensor_copy(
                            d0[0:1, :], dens[32 * par : 32 * par + 1, :]
                        )
                        nc.vector.reciprocal(d0[0:1, :], d0[0:1, :])
                        pbc = ps_s.tile([128, NQ], F32, name="pbc", tag="ps")
                        for half in range(2):
                            hsl = slice(half * 512, (half + 1) * 512)
                            nc.tensor.matmul(
                                pbc[0:D, hsl], lhsT=e0_block[:], rhs=d0[:, hsl],
                                start=True, stop=True,
                            )
                        t1 = small.tile([D, NQ], F32, tag="t1")
                        nc.vector.tensor_tensor(
                            t1[:], av_sb[par * D : par * D + D, :], pbc[0:D, :],
                            mybir.AluOpType.mult,
                        )
                        nc.vector.tensor_tensor(
                            ao[par * D : par * D + D, mt, :],
                            t1[:],
                            bv0[:, h_cur : h_cur + 1].to_broadcast([D, NQ]),
                            mybir.AluOpType.add,
                        )
                # projection
                for od in range(CT):
                    ps = ps_s.tile([128, NQ], F32, name="ps", tag="ps")
                    for half in range(2):
                        hsl = slice(half * 512, (half + 1) * 512)
                        for kc in range(CT):
                            nc.tensor.matmul(
                                ps[:, hsl],
                                lhsT=wproj[:, kc, od * 128 : (od + 1) * 128],
                                rhs=ao[:, kc, hsl],
                                start=(kc == 0),
                                stop=(kc == CT - 1),
                            )
                    o_st = ostp.tile([128, NQ], F32)
                    nc.vector.tensor_tensor(
                        o_st[:],
                        ps[:],
                        bp[:, od : od + 1].to_broadcast([128, NQ]),
                        mybir.AluOpType.add,
                    )
                    nc.sync.dma_start(out=out_r[:, od, :], in_=o_st[:])

    if split:
        _split_sync_waits(nc)
    return nc


_CACHED_NC = None


def _get_nc():
    global _CACHED_NC
    if _CACHED_NC is None:
        _CACHED_NC = build()
    return _CACHED_NC


def make_in_maps(x, w_qkv, b_qkv, w_proj, b_proj):
    bf = ml_dtypes.bfloat16
    wq = np.ascontiguousarray(w_qkv.astype(bf))
    wp = np.ascontiguousarray(w_proj.astype(bf))
    b_q = np.ascontiguousarray(b_qkv[0:C].reshape(C, 1).astype(np.float32))
    b_k = np.ascontiguousarray(b_qkv[C : 2 * C].reshape(C, 1).astype(np.float32))
    b_v0 = np.ascontiguousarray(
        b_qkv[2 * C : 3 * C].reshape(H, D).T.astype(np.float32)
    )
    b_p = np.ascontiguousarray(b_proj.reshape(C, 1).astype(np.float32))

    in_maps = []
    for core in range(NCORES):
        b = core // 2
        qh = core % 2
        xb = x[b]  # [N, C] f32
        xT = np.ascontiguousarray(xb.T.astype(bf))  # [C, N]
        xqT = np.ascontiguousarray(
            xb[qh * NQ : (qh + 1) * NQ].T.astype(bf)
        )  # [C, NQ]
        in_maps.append(
            {
                "xT": xT,
                "xqT": xqT,
                "w_qkv": wq,
                "w_proj": wp,
                "b_q": b_q,
                "b_k": b_k,
                "b_v0": b_v0,
                "b_p": b_p,
            }
        )
    return in_maps


def run(x, w_qkv, b_qkv, w_proj, b_proj, trace=False, **spmd_kwargs):
    nc = _get_nc()
    in_maps = make_in_maps(x, w_qkv, b_qkv, w_proj, b_proj)
    res = run_bass_kernel_spmd(
        nc, in_maps, core_ids=list(range(NCORES)), trace=trace, **spmd_kwargs
    )
    out = np.empty((B, N, C), dtype=np.float32)
    for core in range(NCORES):
        b = core // 2
        qh = core % 2
        yT = res.results[core]["out"]  # [C, NQ] f32
        out[b, qh * NQ : (qh + 1) * NQ, :] = yT.T
    return out, res


def kernel(x, w_qkv, b_qkv, w_proj, b_proj):
    x = np.asarray(x, dtype=np.float32)
    w_qkv = np.asarray(w_qkv, dtype=np.float32)
    b_qkv = np.asarray(b_qkv, dtype=np.float32)
    w_proj = np.asarray(w_proj, dtype=np.float32)
    b_proj = np.asarray(b_proj, dtype=np.float32)
    out, _ = run(x, w_qkv, b_qkv, w_proj, b_proj, trace=False)
    return out


# revision 22
# speedup vs baseline: 1.1666x; 1.0159x over previous
"""Multi-head attention (dense_transformer) on 8 TRN2 NeuronCores.

Decomposition (zero collectives): core c handles batch b = c//2 and query
half qh = c%2.  Each core computes K/V for its batch's full 2048 tokens
(replicated across the 2 cores sharing a batch -- cheaper than any on-chip
collective here), Q for its own 1024 query tokens, all 16 attention heads,
and the output projection for its tokens.  Host does the sharding, the
layout transposes, and the bf16 casts; the NEFF sees only matmul-friendly
layouts.

Layouts on chip (transposed-activation style):
  qT/kT:  [odim (partitions), token (free)]   <- lhsT = w_qkv[c, odim]
  v_aug:  [token%128, token//128, head, 65]   (natural v + ones column;
          the ones column makes the softmax denominator fall out of the
          attn@v matmul as psum row 64)
  scoresT[kt, qt] -> exp on ScalarE (SCALE folded into the activation)
  attn@v: out[d(+den), qt] accumulated over kt tiles
  normalize: reciprocal(den) -> K=1 ones-matmul broadcast -> VectorE
  proj:   yT[odim, qt] = w_proj[c, odim].T @ attnoutT[c, qt]
"""

import sys

if "/opt/trn_rl_repo" not in sys.path:
    sys.path.insert(0, "/opt/trn_rl_repo")

import numpy as np
import ml_dtypes

import concourse.bass as bass
import concourse.mybir as mybir
from concourse.tile import TileContext
from concourse.bass_utils import run_bass_kernel_spmd

F32 = mybir.dt.float32
BF16 = mybir.dt.bfloat16

B = 4
N = 2048
C = 1024
H = 16
D = 64
SCALE = D**-0.5
NQ = N // 2  # query tokens per core
NCORES = 8

CT = C // 128  # 8 c-tiles
TOK_CHUNK = 512
N_CHUNKS = N // TOK_CHUNK  # 4
Q_CHUNKS = NQ // TOK_CHUNK  # 2
KT_TILES = N // 128  # 16

_DMA_TYPES = ("DMA", "Collective", "TriggeredCopy")


def _split_sync_waits(nc, max_waits: int = 1) -> int:
    """This container's walrus rejects TPB instructions with >1 sync-wait;
    hoist extras onto InstNoOps inserted just before, on the same engine."""
    n_split = 0
    for fn in nc.m.functions:
        for block in fn.blocks:
            out = []
            changed = False
            for inst in block.instructions:
                tname = type(inst).__name__
                si = getattr(inst, "sync_info", None)
                if si is not None and len(si.on_wait) > max_waits:
                    waits = list(si.on_wait)
                    n_extra = len(waits) - max_waits
                    for i in range(0, n_extra, max_waits):
                        out.append(
                            mybir.InstNoOp(
                                name=f"{inst.name}-sw{i}",
                                sync_info=mybir.SyncInfo(
                                    on_wait=waits[i : i + max_waits], on_update=[]
                                ),
                                bass_nofuse=True,
                                engine=inst.engine,
                            )
                        )
                    inst.sync_info = mybir.SyncInfo(
                        on_wait=waits[n_extra:], on_update=list(si.on_update)
                    )
                    changed = True
                    n_split += 1
                out.append(inst)
            if changed:
                block.instructions = out
    return n_split


def build(split=True, n_chunks=N_CHUNKS, q_chunks=1, n_heads=H, kt_tiles=KT_TILES):
    nc = bass.Bass(target_bir_lowering=False)

    xT_ext = nc.declare_dram_parameter("xT", [C, N], BF16, isOutput=False)
    xqT_ext = nc.declare_dram_parameter("xqT", [C, NQ], BF16, isOutput=False)
    wqkv_ext = nc.declare_dram_parameter("w_qkv", [C, 3 * C], BF16, isOutput=False)
    wproj_ext = nc.declare_dram_parameter("w_proj", [C, C], BF16, isOutput=False)
    bq_ext = nc.declare_dram_parameter("b_q", [C, 1], F32, isOutput=False)
    bk_ext = nc.declare_dram_parameter("b_k", [C, 1], F32, isOutput=False)
    bv0_ext = nc.declare_dram_parameter("b_v0", [D, H], F32, isOutput=False)
    bp_ext = nc.declare_dram_parameter("b_p", [C, 1], F32, isOutput=False)
    out_ext = nc.declare_dram_parameter("out", [C, NQ], F32, isOutput=True)

    xT_r = xT_ext[:].rearrange("(o p) n -> p o n", p=128)
    xqT_r = xqT_ext[:].rearrange("(o p) n -> p o n", p=128)
    out_r = out_ext[:].rearrange("(o p) n -> p o n", p=128)

    with TileContext(nc) as tc:
        with (
            tc.tile_pool(name="const", bufs=1) as const,
            tc.tile_pool(name="xin", bufs=2) as xin,
            tc.tile_pool(name="big", bufs=1) as big,
            tc.tile_pool(name="attn", bufs=3) as attnp,
            tc.tile_pool(name="ao", bufs=1) as aop,
            tc.tile_pool(name="small", bufs=2) as small,
            tc.tile_pool(name="ost", bufs=1) as ostp,
            tc.tile_pool(name="ps_s", bufs=2, space="PSUM") as ps_s,
            tc.tile_pool(name="ps_av", bufs=2, space="PSUM") as ps_av,
        ):
            wqkv = const.tile([128, CT, 3 * C], BF16)
            wproj = const.tile([128, CT, C], BF16)
            bq = const.tile([128, CT], F32)
            bk = const.tile([128, CT], F32)
            bv0 = const.tile([D, H], F32)
            bp = const.tile([128, CT], F32)
            e0_block = const.tile([128, D], F32)
            d0 = const.tile([128, NQ], F32)

            kT = big.tile([128, CT, N], BF16)
            qT = big.tile([128, CT, NQ], BF16)
            v_aug = big.tile([128, KT_TILES, H, D + 1], BF16)

            wqkv_r = wqkv_ext[:].rearrange("(o p) n -> p o n", p=128)
            wproj_r = wproj_ext[:].rearrange("(o p) n -> p o n", p=128)
            for kc in range(CT):
                nc.sync.dma_start(out=wqkv[:, kc : kc + 1, :], in_=wqkv_r[:, kc : kc + 1, :])
            for kc in range(CT):
                nc.sync.dma_start(out=wproj[:, kc : kc + 1, :], in_=wproj_r[:, kc : kc + 1, :])
            nc.sync.dma_start(out=bq[:], in_=bq_ext[:].rearrange("(o p) 1 -> p o", p=128))
            nc.sync.dma_start(out=bk[:], in_=bk_ext[:].rearrange("(o p) 1 -> p o", p=128))
            nc.sync.dma_start(out=bv0[:], in_=bv0_ext[:])
            nc.sync.dma_start(out=bp[:], in_=bp_ext[:].rearrange("(o p) 1 -> p o", p=128))
            nc.vector.memset(e0_block[:], 0.0)
            nc.vector.memset(e0_block[0:1, :], 1.0)
            nc.vector.memset(d0[:], 0.0)
            for h in range(H):
                nc.vector.memset(v_aug[:, :, h, D : D + 1], 1.0)

            # ---- Phase B: qkv projections -------------------------------
            for t in range(n_chunks):
                x_c = xin.tile([128, CT, TOK_CHUNK], BF16, tag="xc")
                nc.sync.dma_start(
                    out=x_c[:], in_=xT_r[:, :, t * TOK_CHUNK : (t + 1) * TOK_CHUNK]
                )
                # kT for this token chunk (two odim tiles per 2-bank psum tile)
                for m2 in range(CT // 2):
                    ps = ps_s.tile([128, NQ], F32, name="ps", tag="ps")
                    for sub in range(2):
                        m = m2 * 2 + sub
                        for kc in range(CT):
                            nc.tensor.matmul(
                                ps[:, sub * 512 : (sub + 1) * 512],
                                lhsT=wqkv[:, kc, C + m * 128 : C + (m + 1) * 128],
                                rhs=x_c[:, kc, :],
                                start=(kc == 0),
                                stop=(kc == CT - 1),
                            )
                    nc.vector.tensor_tensor(
                        kT[:, m2 * 2 : m2 * 2 + 2, t * TOK_CHUNK : (t + 1) * TOK_CHUNK],
                        ps[:].rearrange("p (s n) -> p s n", s=2),
                        bk[:, m2 * 2 : m2 * 2 + 2, None].to_broadcast([128, 2, TOK_CHUNK]),
                        mybir.AluOpType.add,
                    )
                # v (natural layout) for this token chunk
                for tt in range(TOK_CHUNK // 128):
                    kt_idx = t * (TOK_CHUNK // 128) + tt
                    ps = ps_s.tile([128, NQ], F32, name="ps", tag="ps")
                    for vc in range(2):
                        for kc in range(CT):
                            nc.tensor.matmul(
                                ps[:, vc * 512 : (vc + 1) * 512],
                                lhsT=x_c[:, kc, tt * 128 : (tt + 1) * 128],
                                rhs=wqkv[:, kc, 2 * C + vc * 512 : 2 * C + (vc + 1) * 512],
                                start=(kc == 0),
                                stop=(kc == CT - 1),
                            )
                    nc.vector.tensor_copy(
                        v_aug[:, kt_idx, :, 0:D],
                        ps[:].rearrange("p (h d) -> p h d", d=D),
                    )
            # qT for this core's query tokens
            for tq in range(NQ // TOK_CHUNK):
                xq_c = xin.tile([128, CT, TOK_CHUNK], BF16, tag="xc")
                nc.sync.dma_start(
                    out=xq_c[:], in_=xqT_r[:, :, tq * TOK_CHUNK : (tq + 1) * TOK_CHUNK]
                )
                for m2 in range(CT // 2):
                    ps = ps_s.tile([128, NQ], F32, name="ps", tag="ps")
                    for sub in range(2):
                        m = m2 * 2 + sub
                        for kc in range(CT):
                            nc.tensor.matmul(
                                ps[:, sub * 512 : (sub + 1) * 512],
                                lhsT=wqkv[:, kc, m * 128 : (m + 1) * 128],
                                rhs=xq_c[:, kc, :],
                                start=(kc == 0),
                                stop=(kc == CT - 1),
                            )
                    nc.vector.tensor_tensor(
                        qT[:, m2 * 2 : m2 * 2 + 2, tq * TOK_CHUNK : (tq + 1) * TOK_CHUNK],
                        ps[:].rearrange("p (s n) -> p s n", s=2),
                        bq[:, m2 * 2 : m2 * 2 + 2, None].to_broadcast([128, 2, TOK_CHUNK]),
                        mybir.AluOpType.add,
                    )

            # ---- Phase C: attention + projection (head pairs, full-array MMs) ----
            # K=64 matmuls run at half clock AND poison neighbors, so the two
            # heads sharing an odim tile (partitions 0:64 / 64:128) are issued
            # as concurrent row-group pairs; attn@V pairs are col-packed
            # (M=64 each) into one psum tile; softmax denominators come from
            # K=128 ones-column matmuls; the reciprocal-broadcast is a K=128
            # matmul against a zero-padded row (avoids cold K=1 matmuls).
            for Q in range(q_chunks):
                ao = aop.tile([128, CT, NQ], BF16)
                for pair in range(n_heads // 2):
                    mt = pair
                    h_e, h_o = 2 * pair, 2 * pair + 1
                    pav_e = ps_av.tile([128, NQ], F32, name="pav_e", tag="pav")
                    pav_o = ps_av.tile([128, NQ], F32, name="pav_o", tag="pav")
                    for kt in range(kt_tiles):
                        pss_e = ps_s.tile([128, NQ], F32, name="pss_e", tag="ps")
                        pss_o = ps_s.tile([128, NQ], F32, name="pss_o", tag="ps")
                        for half in range(2):
                            hsl = slice(half * 512, (half + 1) * 512)
                            nc.tensor.matmul(
                                pss_e[:, hsl],
                                lhsT=kT[0:D, mt, kt * 128 : (kt + 1) * 128],
                                rhs=qT[0:D, mt, hsl],
                                start=True,
                                stop=True,
                            )
                            nc.tensor.matmul(
                                pss_o[:, hsl],
                                lhsT=kT[D:128, mt, kt * 128 : (kt + 1) * 128],
                                rhs=qT[D:128, mt, hsl],
                                start=True,
                                stop=True,
                            )
                        at_e = attnp.tile([128, NQ], BF16, name="at_e", tag="at")
                        at_o = attnp.tile([128, NQ], BF16, name="at_o", tag="at")
                        nc.scalar.activation(
                            at_e[:], pss_e[:],
                            mybir.ActivationFunctionType.Exp, scale=float(SCALE),
                        )
                        nc.scalar.activation(
                            at_o[:], pss_o[:],
                            mybir.ActivationFunctionType.Exp, scale=float(SCALE),
                        )
                        first, last = kt == 0, kt == kt_tiles - 1
                        for half in range(2):
                            hsl = slice(half * 512, (half + 1) * 512)
                            nc.tensor.matmul(
                                pav_e[0 : D + 1, hsl],
                                lhsT=v_aug[:, kt, h_e, :],
                                rhs=at_e[:, hsl],
                                start=first, stop=last,
                            )
                            nc.tensor.matmul(
                                pav_o[0 : D + 1, hsl],
                                lhsT=v_aug[:, kt, h_o, :],
                                rhs=at_o[:, hsl],
                                start=first, stop=last,
                            )
                    # normalize both heads of the pair
                    for par, (h_cur, pav) in enumerate(((h_e, pav_e), (h_o, pav_o))):
                        av_sb = small.tile([128, NQ], F32, tag="av")
                        nc.vector.tensor_copy(av_sb[0 : D + 1, :], pav[0 : D + 1, :])
                        nc.vector.tensor_copy(d0[0:1, :], av_sb[D : D + 1, :])
                        nc.vector.reciprocal(d0[0:1, :], d0[0:1, :])
                        pbc = ps_s.tile([128, NQ], F32, name="pbc", tag="ps")
                        for half in range(2):
                            hsl = slice(half * 512, (half + 1) * 512)
                            nc.tensor.matmul(
                                pbc[0:D, hsl], lhsT=e0_block[:], rhs=d0[:, hsl],
                                start=True, stop=True,
                            )
                        t1 = small.tile([D, NQ], F32, tag="t1")
                        nc.vector.tensor_tensor(
                            t1[:], av_sb[0:D, :], pbc[0:D, :],
                            mybir.AluOpType.mult,
                        )
                        nc.vector.tensor_tensor(
                            ao[par * D : par * D + D, mt, :],
                            t1[:],
                            bv0[:, h_cur : h_cur + 1].to_broadcast([D, NQ]),
                            mybir.AluOpType.add,
                        )
                # projection
                for od in range(CT):
                    ps = ps_s.tile([128, NQ], F32, name="ps", tag="ps")
                    for half in range(2):
                        hsl = slice(half * 512, (half + 1) * 512)
                        for kc in range(CT):
                            nc.tensor.matmul(
                                ps[:, hsl],
                                lhsT=wproj[:, kc, od * 128 : (od + 1) * 128],
                                rhs=ao[:, kc, hsl],
                                start=(kc == 0),
                                stop=(kc == CT - 1),
                            )
                    o_st = ostp.tile([128, NQ], F32)
                    nc.vector.tensor_tensor(
                        o_st[:],
                        ps[:],
                        bp[:, od : od + 1].to_broadcast([128, NQ]),
                        mybir.AluOpType.add,
                    )
                    nc.sync.dma_start(out=out_r[:, od, :], in_=o_st[:])

    if split:
        _split_sync_waits(nc)
    return nc


_CACHED_NC = None


def _get_nc():
    global _CACHED_NC
    if _CACHED_NC is None:
        _CACHED_NC = build()
    return _CACHED_NC


def make_in_maps(x, w_qkv, b_qkv, w_proj, b_proj):
    bf = ml_dtypes.bfloat16
    wq = np.ascontiguousarray(w_qkv.astype(bf))
    wp = np.ascontiguousarray(w_proj.astype(bf))
    b_q = np.ascontiguousarray(b_qkv[0:C].reshape(C, 1).astype(np.float32))
    b_k = np.ascontiguousarray(b_qkv[C : 2 * C].reshape(C, 1).astype(np.float32))
    b_v0 = np.ascontiguousarray(
        b_qkv[2 * C : 3 * C].reshape(H, D).T.astype(np.float32)
    )
    b_p = np.ascontiguousarray(b_proj.reshape(C, 1).astype(np.float32))

    in_maps = []
    for core in range(NCORES):
        b = core // 2
        qh = core % 2
        xb = x[b]  # [N, C] f32
        xT = np.ascontiguousarray(xb.T.astype(bf))  # [C, N]
        xqT = np.ascontiguousarray(
            xb[qh * NQ : (qh + 1) * NQ].T.astype(bf)
        )  # [C, NQ]
        in_maps.append(
            {
                "xT": xT,
                "xqT": xqT,
                "w_qkv": wq,
                "w_proj": wp,
                "b_q": b_q,
                "b_k": b_k,
                "b_v0": b_v0,
                "b_p": b_p,
            }
        )
    return in_maps


def run(x, w_qkv, b_qkv, w_proj, b_proj, trace=False, **spmd_kwargs):
    nc = _get_nc()
    in_maps = make_in_maps(x, w_qkv, b_qkv, w_proj, b_proj)
    res = run_bass_kernel_spmd(
        nc, in_maps, core_ids=list(range(NCORES)), trace=trace, **spmd_kwargs
    )
    out = np.empty((B, N, C), dtype=np.float32)
    for core in range(NCORES):
        b = core // 2
        qh = core % 2
        yT = res.results[core]["out"]  # [C, NQ] f32
        out[b, qh * NQ : (qh + 1) * NQ, :] = yT.T
    return out, res


def kernel(x, w_qkv, b_qkv, w_proj, b_proj):
    x = np.asarray(x, dtype=np.float32)
    w_qkv = np.asarray(w_qkv, dtype=np.float32)
    b_qkv = np.asarray(b_qkv, dtype=np.float32)
    w_proj = np.asarray(w_proj, dtype=np.float32)
    b_proj = np.asarray(b_proj, dtype=np.float32)
    out, _ = run(x, w_qkv, b_qkv, w_proj, b_proj, trace=False)
    return out
